# revision 77
# baseline (speedup 1.0000x reference)
"""MoE layer (8 experts, top-2, SwiGLU FFN) on 8 Trainium2 NeuronCores.

Strategy: expert parallelism. Each core owns one expert's weights (bf16)
and redundantly computes the fp32 router (cheap). Token dispatch is done
with the gpsimd dma_gather instruction (indirect DMA straight into the
transposed [h, slot] activation layout) instead of a one-hot matmul, so
the tensor engine only runs the FFN itself. The device returns the
per-slot expert outputs yT[H, CAP] plus the routing info (slot + weight
per token); the host applies the routing weights and scatter-adds the 8
cores' slots back to token order (the expert "combine").
"""

import numpy as np
import ml_dtypes

import concourse.mybir as mybir
import concourse.tile as tile
from concourse import bacc
from concourse import library_config

F32 = mybir.dt.float32
BF16 = mybir.dt.bfloat16
I16 = mybir.dt.int16
F16 = mybir.dt.float16
AT = mybir.ActivationFunctionType
OP = mybir.AluOpType

# Problem sizes (fixed by the reference model)
B, S, H, FF, E = 2, 1024, 1024, 4096, 8
T = B * S                       # 2048 tokens
CAPG = 640                      # slot space (multiple of 128; max count 540)
CAPF = 540                      # FFN capacity actually computed (2x270 chunks)
BIG = 65536.0                   # "no slot" marker; exact fp32 round-trip
WHOLD = 0.026                   # ms: hold FFN weight stream off the DMA
                                # queues until the router's xT load is done


def _chunks(total, step):
    out, o = [], 0
    while o < total:
        out.append((o, min(step, total - o)))
        o += step
    return out


def build_nc(T=T, H=H, FF=FF, E=E, CAPG=CAPG, CAPF=CAPF):
    NT, NH, NF = T // 128, H // 128, FF // 128
    NCG = CAPG // 128           # slot tiles (128-wide) in slot space
    # equal-split FFN capacity chunks <=512 keep matmuls compute-bound
    ncch = (CAPF + 511) // 512
    CCH = _chunks(CAPF, -(-CAPF // ncch))

    nc = bacc.Bacc("TRN2", target_bir_lowering=False, debug=False)

    xT = nc.dram_tensor("xT", [H, T], F32, kind="ExternalInput")
    xtok = nc.dram_tensor("xtok", [T, H], BF16, kind="ExternalInput")
    wrT = nc.dram_tensor("wrT", [128, H // 128, E], F32, kind="ExternalInput")
    sel8 = nc.dram_tensor("sel8", [128, E], F32, kind="ExternalInput")
    w1r = nc.dram_tensor("w1r", [NF, 128, NH, 128], BF16, kind="ExternalInput")
    w3r = nc.dram_tensor("w3r", [NF, 128, NH, 128], BF16, kind="ExternalInput")
    w2r = nc.dram_tensor("w2r", [FF, H], BF16, kind="ExternalInput")
    iotaC = nc.dram_tensor("iotaC", [128, CAPG], F16, kind="ExternalInput")
    uincl = nc.dram_tensor("uincl", [128, 128], F32, kind="ExternalInput")
    onesc = nc.dram_tensor("onesc", [128, 128], F32, kind="ExternalInput")
    identf = nc.dram_tensor("identf", [128, 128], F32, kind="ExternalInput")
    # lhsT columns for the slot->token matmul: col 2*tt = partition index p,
    # col 2*tt+1 = tt*128 (both exact in bf16)
    ptt2 = nc.dram_tensor("ptt2", [128, 2 * NT], BF16, kind="ExternalInput")
    # partition-permutation matrices for the idx 16-wrap:
    # permw[c, k, p] = 1 iff c == k*16 + p%16
    permw = nc.dram_tensor("permw", [128, 8, 128], F32, kind="ExternalInput")

    yTo = nc.dram_tensor("yT", [H, CAPF], BF16, kind="ExternalOutput")
    w16o = nc.dram_tensor("w16o", [128, NT], F32, kind="ExternalOutput")
    s16o = nc.dram_tensor("s16o", [128, NT], F32, kind="ExternalOutput")

    with tile.TileContext(nc) as tc:
        with (
            tc.tile_pool(name="const", bufs=1) as constp,
            tc.tile_pool(name="pers", bufs=1) as pers,
            tc.tile_pool(name="stream", bufs=2) as streamp,
            tc.tile_pool(name="wstream", bufs=4) as wstream,
            tc.tile_pool(name="outp", bufs=4) as outp,
        ):
            nc.gpsimd.load_library(library_config.mlp)

            # ---- constants ----
            # only the router-critical wrT goes first; the rest are issued
            # after the router's xT DMAs so they don't delay the front
            wrT_sb = constp.tile([128, NH, E], F32)
            nc.sync.dma_start(wrT_sb, wrT[:])
            sel_sb = constp.tile([128, E], F32)
            nc.sync.dma_start(sel_sb, sel8[:])
            # dummy op: pull the scalar engine's Exp table load (~1.3us)
            # into the idle xT-load window instead of paying it on the
            # top-2 critical path at the first real Exp
            dume = streamp.tile([1, 2], F32, tag="dume")
            nc.scalar.activation(dume, sel_sb[0:1, 0:2], AT.Exp)
            iota_sb = constp.tile([128, CAPG], F16)
            u_sb = constp.tile([128, 128], F32)
            ones_sb = constp.tile([128, 128], F32)
            idf_sb = constp.tile([128, 128], F32)
            ptt2_sb = constp.tile([128, 2 * NT], BF16)
            permw_sb = constp.tile([128, 8, 128], F32)

            le16 = pers.tile([128, NT], F32)     # own-expert logit
            max8_sb = pers.tile([128, NT, 8], F32)
            m16 = pers.tile([128, NT], F32)
            w16 = pers.tile([128, NT], F32)
            s16 = pers.tile([128, NT], F32)
            xgT = pers.tile([128, NH, CAPG], BF16)
            hmid = pers.tile([128, NF, CAPF], BF16)
            idxs_sb = pers.tile([128, CAPG // 16], I16)

            with tc.tile_pool(name="ps_small", bufs=6,
                              space="PSUM") as ps_small:
                # ---- router (fp32): logitsT[E, T], WrT stationary ----
                # full xT rows per DMA (8KB/partition) for DMA efficiency;
                # token chunks become interleaved psum groups
                lgT_sb = pers.tile([E, T], F32)
                TCH = _chunks(T, 512)
                ps_lrs = [ps_small.tile([128, 512], F32, tag="small",
                                        name=f"pslr{i}")
                          for i in range(len(TCH))]
                with tc.tile_pool(name="xtfp", bufs=3) as xtfp:
                    for ht in range(NH):
                        xtf = xtfp.tile([128, T], F32, tag="xtf")
                        # split into 1024-column pieces: one monolithic 1MB
                        # DMA is serviced by a single engine (~45us), while
                        # narrower splits shrink the per-partition
                        # descriptor below 4KB and halve DMA efficiency.
                        # 16 pieces of 512KB match the 16 engines.
                        for to in range(0, T, 1024):
                            nc.sync.dma_start(
                                xtf[:, to:to + 1024],
                                xT[ht * 128:(ht + 1) * 128, to:to + 1024])
                        if ht == 0:
                            # non-critical const loads, after first xT
                            nc.sync.dma_start(iota_sb, iotaC[:])
                            nc.sync.dma_start(u_sb, uincl[:])
                            nc.sync.dma_start(ones_sb, onesc[:])
                            nc.sync.dma_start(idf_sb, identf[:])
                            nc.sync.dma_start(ptt2_sb, ptt2[:])
                            nc.sync.dma_start(permw_sb, permw[:])
                        for i, (to, ts_) in enumerate(TCH):
                            nc.tensor.matmul(ps_lrs[i][:E, :ts_],
                                             lhsT=wrT_sb[:, ht, :],
                                             rhs=xtf[:, to:to + ts_],
                                             start=(ht == 0),
                                             stop=(ht == NH - 1))
                for i, (to, ts_) in enumerate(TCH):
                    nc.scalar.copy(lgT_sb[:, to:to + ts_],
                                   ps_lrs[i][:E, :ts_])
                # prefetch the first FFN1 weight tiles ahead of the
                # dispatch phase
                pre_w = []
                for ft in range(2):
                    w1t = wstream.tile([128, NH, 128], BF16, tag="w1t")
                    nc.sync.dma_start(w1t, w1r[ft])
                    w3t = wstream.tile([128, NH, 128], BF16, tag="w3t")
                    nc.sync.dma_start(w3t, w3r[ft])
                    pre_w.append((w1t, w3t))
                # transpose logitsT back to [token_p, E] per tile
                for tt in range(NT):
                    ps_lt = ps_small.tile([128, 128], F32, tag="small")
                    nc.tensor.transpose(
                        ps_lt[:, :E],
                        lgT_sb[:, tt * 128:(tt + 1) * 128],
                        idf_sb[:E, :E])
                    lg = streamp.tile([128, E], F32, tag="lg")
                    nc.scalar.copy(lg, ps_lt[:, :E])
                    nc.vector.max(max8_sb[:, tt, :], lg)
                    tmp8 = streamp.tile([128, E], F32, tag="tmp8")
                    nc.vector.tensor_mul(tmp8, lg, sel_sb)
                    nc.vector.tensor_reduce(
                        le16[:, tt:tt + 1], tmp8, mybir.AxisListType.X,
                        OP.add)
                    # m16 computed per tile inside the DMA-paced router
                    # loop: the cumsum below then starts immediately after
                    # the last tile, with no batched chain in front of it
                    nc.vector.tensor_tensor(
                        m16[:, tt:tt + 1], le16[:, tt:tt + 1],
                        max8_sb[:, tt, 1:2], OP.is_ge)

                # ---- slot assignment: cumsum of mask over tokens ----
                ps_cs = ps_small.tile([128, 128], F32, tag="small")
                nc.tensor.matmul(ps_cs[:, :NT], lhsT=u_sb, rhs=m16,
                                 start=True, stop=True)
                ps_tot = ps_small.tile([128, 128], F32, tag="small")
                nc.tensor.matmul(ps_tot[:, :NT], lhsT=ones_sb, rhs=m16,
                                 start=True, stop=True)
                tot_sb = pers.tile([128, NT], F32)
                nc.scalar.copy(tot_sb, ps_tot[:, :NT])
                isc1 = pers.tile([128, NT], F32)
                nc.vector.tensor_tensor_scan(
                    out=isc1, data0=tot_sb, data1=ones_sb[:, :NT],
                    initial=-1.0, op0=OP.add, op1=OP.mult)
                carrym1 = pers.tile([128, NT], F32)
                nc.vector.tensor_sub(carrym1, isc1, tot_sb)
                s_a = pers.tile([128, NT], F32)
                nc.vector.tensor_tensor(s_a, ps_cs[:, :NT], carrym1, OP.add)
                # s16 = m16 ? s_a : BIG  ==  (s_a - BIG)*m16 + BIG
                # (exact fp32 arithmetic, fused into two DVE ops)
                nc.vector.scalar_tensor_tensor(s_a, s_a, BIG, m16,
                                               OP.subtract, OP.mult)
                nc.vector.tensor_scalar(s16, s_a, BIG, None, OP.add)
                nc.sync.dma_start(s16o[:], s16)

                # ---- slot -> token index vector (exact int arithmetic) ----
                # one-hot dispatch matrix St[tok_p, tile, slot], then
                # tok(c) = sum_t p*St + sum_t (tt*128)*St via 2-col matmuls
                with tc.tile_pool(name="stp", bufs=1) as stp:
                    St = stp.tile([128, NT, CAPG], BF16)
                    # slots beyond CAPF never occur (max count <= CAPF by
                    # the host capacity check), so the one-hot / slot->token
                    # work only covers CAPF of the CAPG slot space
                    for tt in range(NT):
                        # fp16 iota halves the dominant DVE read
                        # traffic on this serial chain (ids exact in fp16)
                        nc.vector.tensor_scalar(
                            St[:, tt, :CAPF], iota_sb[:, :CAPF],
                            s16[:, tt:tt + 1], None, OP.is_equal)
                    TKCH = _chunks(CAPF, 512)
                    ps_toks = [ps_small.tile([2, 512], F32, tag="small",
                                             name=f"pstok{i}")
                               for i in range(len(TKCH))]
                    for tt in range(NT):
                        for i, (co, cs) in enumerate(TKCH):
                            nc.tensor.matmul(
                                ps_toks[i][:, :cs],
                                lhsT=ptt2_sb[:, 2 * tt:2 * tt + 2],
                                rhs=St[:, tt, co:co + cs],
                                start=(tt == 0), stop=(tt == NT - 1))
                    tok2 = pers.tile([2, CAPG], F32)
                    # the whole dead tail must be zeroed: NaN garbage in
                    # any tokc row poisons the permutation matmuls
                    # (0*NaN=NaN spreads across the wrap column). Split at
                    # the 16-boundary into two known-legal memset widths.
                    pad16 = -(-CAPF // 16) * 16
                    if pad16 > CAPF:
                        nc.vector.memset(tok2[:, CAPF:pad16], 0.0)
                    if CAPG > pad16:
                        nc.vector.memset(tok2[:, pad16:], 0.0)
                    for i, (co, cs) in enumerate(TKCH):
                        nc.scalar.copy(tok2[:, co:co + cs],
                                       ps_toks[i][:, :cs])
                # transpose [2, CAPG] -> [128, NCG, 2]; add the two columns.
                # All transposes land in one PSUM tile so the copy and the
                # strided add are single instructions (this chain is
                # instruction-latency-bound, not throughput-bound)
                tokc = pers.tile([128, NCG], F32)
                ps_tt = ps_small.tile([128, NCG, 2], F32, tag="small")
                for ct in range(NCG):
                    nc.tensor.transpose(
                        ps_tt[:, ct, :], tok2[:, ct * 128:(ct + 1) * 128],
                        idf_sb[:2, :2])
                tk2 = streamp.tile([128, NCG, 2], F32, tag="tk2")
                nc.scalar.copy(tk2, ps_tt)
                nc.vector.tensor_tensor(
                    tokc, tk2[:, :, 0], tk2[:, :, 1], OP.add)
                # wrap to the gpsimd idx layout ([16, CAPG/16] wrapped,
                # replicated on all 128 partitions) ON-CHIP via 8 constant
                # partition-permutation matmuls (exact fp32 0/1). A DRAM
                # round-trip here would ride the DMA engines, which are
                # saturated by the weight stream at this point (~30us stall)
                for k in range(8):
                    ps_pk = ps_small.tile([128, NCG], F32, tag="small")
                    nc.tensor.matmul(ps_pk, lhsT=permw_sb[:, k, :],
                                     rhs=tokc, start=True, stop=True)
                    # cast straight from PSUM into the strided int16
                    # slots, on the otherwise-idle scalar engine
                    nc.scalar.copy(idxs_sb[:, k::8], ps_pk)
                # slots >= CAPF get idx -1: the gather stops at the last
                # non-negative index, skipping ~15% of descriptor
                # generation and transfer for the dead slot-space tail
                nwrap = -(-CAPF // 16)
                nc.vector.memset(idxs_sb[:, nwrap:], -1.0)

                # ---- top-2 softmax weights (off the critical path: they
                # only feed the w16o output, so they run while the gather
                # descriptor generation proceeds on gpsimd) ----
                l1 = max8_sb[:, :, 0]
                l2 = max8_sb[:, :, 1]
                d_e = pers.tile([128, NT], F32)
                nc.vector.tensor_sub(d_e, le16, l1)
                e_e = pers.tile([128, NT], F32)
                nc.scalar.activation(e_e, d_e, AT.Exp)
                d_2 = pers.tile([128, NT], F32)
                nc.vector.tensor_sub(d_2, l2, l1)
                e_2 = pers.tile([128, NT], F32)
                # (sigmoid would be one op, but switching the scalar
                # engine's activation table Exp->Sigmoid costs ~1.3us)
                nc.scalar.activation(e_2, d_2, AT.Exp)
                nc.vector.tensor_scalar_add(e_2, e_2, 1.0)
                rden = pers.tile([128, NT], F32)
                nc.vector.reciprocal(rden, e_2)
                nc.vector.tensor_mul(w16, e_e, rden)
                nc.vector.tensor_mul(w16, w16, m16)
                nc.sync.dma_start(w16o[:], w16)
                # dummy op: pull the scalar engine's Sigmoid table load
                # (~1.3us) into the idle dispatch window instead of paying
                # it at FFN1's first real sigmoid
                dumw = streamp.tile([1, 2], F32, tag="dumw")
                nc.scalar.activation(dumw, w16[0:1, 0:2], AT.Sigmoid)

            # ---- token gather: xgT[h, c] = x[tok(c), h] via indirect DMA --
            nc.gpsimd.dma_gather(xgT[:], xtok[:, :], idxs_sb[:],
                                 CAPG, nwrap * 16, H, transpose=True)

            # ---- FFN part 1 + W2 residency prefetch ----
            with tc.tile_pool(name="w2pool", bufs=1) as w2pool:
                w2res = w2pool.tile([128, NF, H], BF16)
                w2rr = w2r.rearrange("(n p) h -> p n h", p=128)
                with (
                    tc.tile_pool(name="ps_gate", bufs=2,
                                 space="PSUM") as ps_gate,
                    tc.tile_pool(name="ps_up", bufs=2, space="PSUM") as ps_up,
                    # ps_y opened alongside (7 of 8 banks total): a
                    # close/reopen barrier between FFN1 and FFN2 costs ~1.2us
                    tc.tile_pool(name="ps_y", bufs=3, space="PSUM") as ps_y,
                ):
                    for ft in range(NF):
                        if ft < len(pre_w):
                            w1t, w3t = pre_w[ft]
                        else:
                            w1t = wstream.tile([128, NH, 128], BF16,
                                               tag="w1t")
                            w3t = wstream.tile([128, NH, 128], BF16,
                                               tag="w3t")
                            # Gate the stream behind the token gather: a
                            # dummy gpsimd write into the target tile reads
                            # xgT (ready only once the gather DMA landed)
                            # and the DMA's WAW dep on it holds the weight
                            # traffic off the DMA engines until then.
                            # Ungated, weights steal ~half the bandwidth
                            # from the router's xT load and the gather,
                            # pushing FFN1's start out ~28us. The stream
                            # still finishes well before FFN2 needs W2.
                            if ft < 6:
                                nc.gpsimd.tensor_copy(w1t[0:E, 0, 0:1],
                                                      xgT[0:E, 0, 0:1])
                                nc.gpsimd.tensor_copy(w3t[0:E, 0, 0:1],
                                                      xgT[0:E, 0, 0:1])
                            nc.sync.dma_start(w1t, w1r[ft])
                            nc.sync.dma_start(w3t, w3r[ft])
                        nc.gpsimd.tensor_copy(w2res[0:E, ft, 0:1],
                                              xgT[0:E, 0, 0:1])
                        # interleave the W2 residency load with the stream
                        nc.sync.dma_start(w2res[:, ft, :],
                                          w2rr[:, ft, :])
                        for (co, cs) in CCH:
                            psg = ps_gate.tile([128, 512], F32, tag="gate")
                            psu = ps_up.tile([128, 512], F32, tag="up")
                            for ht in range(NH):
                                nc.tensor.matmul(
                                    psg[:, :cs], lhsT=w1t[:, ht, :],
                                    rhs=xgT[:, ht, co:co + cs],
                                    start=(ht == 0), stop=(ht == NH - 1))
                            for ht in range(NH):
                                nc.tensor.matmul(
                                    psu[:, :cs], lhsT=w3t[:, ht, :],
                                    rhs=xgT[:, ht, co:co + cs],
                                    start=(ht == 0), stop=(ht == NH - 1))
                            sil = streamp.tile([128, 512], F32, tag="sil")
                            nc.scalar.activation(sil[:, :cs], psg[:, :cs],
                                                 AT.Sigmoid)
                            tmp = streamp.tile([128, 512], F32, tag="ftmp")
                            nc.vector.tensor_mul(tmp[:, :cs], sil[:, :cs],
                                                 psu[:, :cs])
                            nc.vector.tensor_mul(hmid[:, ft, co:co + cs],
                                                 tmp[:, :cs], psg[:, :cs])

                    # ---- FFN part 2: yT[h, c] = sum_f W2[f, h] hmid[f, c] --
                    yTr = yTo.rearrange("(n p) c -> p n c", p=128)
                    ngrp = len(CCH) * NH
                    for gi, ((co, cs), ht) in enumerate(
                            (c, h) for c in CCH for h in range(NH)):
                        psy = ps_y.tile([128, 512], F32, tag="y")
                        for ft in range(NF):
                            nc.tensor.matmul(
                                psy[:, :cs],
                                lhsT=w2res[:, ft,
                                           ht * 128:(ht + 1) * 128],
                                rhs=hmid[:, ft, co:co + cs],
                                start=(ft == 0), stop=(ft == NF - 1))
                        ysb = outp.tile([128, 512], BF16, tag="ysb")
                        nc.scalar.copy(ysb[:, :cs], psy[:, :cs])
                        nc.sync.dma_start(yTr[:, ht, co:co + cs],
                                          ysb[:, :cs])

    nc.compile()
    return nc


_NC_CACHE = {}


def _get_nc(key=(T, H, FF, E, CAPG, CAPF)):
    if key not in _NC_CACHE:
        _NC_CACHE[key] = build_nc(*key)
    return _NC_CACHE[key]


def make_in_maps(x, Wr, W1, W2, W3, T=T, H=H, FF=FF, E=E, CAPG=CAPG):
    NT, NH, NF = T // 128, H // 128, FF // 128
    bf = ml_dtypes.bfloat16
    xf = np.ascontiguousarray(x.reshape(T, H)).astype(np.float32)
    ptt2 = np.zeros((128, 2 * NT), dtype=np.float32)
    ptt2[:, 0::2] = np.arange(128, dtype=np.float32)[:, None]
    ptt2[:, 1::2] = 128.0 * np.arange(NT, dtype=np.float32)[None, :]
    permw_np = np.zeros((128, 8, 128), dtype=np.float32)
    for k in range(8):
        for p in range(128):
            permw_np[k * 16 + p % 16, k, p] = 1.0
    base = {
        "xT": np.ascontiguousarray(xf.T),
        "xtok": xf.astype(bf),
        "wrT": np.ascontiguousarray(
            np.asarray(Wr, dtype=np.float32).T.reshape(H // 128, 128, -1)
            .transpose(1, 0, 2)),
        "iotaC": np.ascontiguousarray(
            np.tile(np.arange(CAPG, dtype=np.float16), (128, 1))),
        "uincl": np.triu(np.ones((128, 128), dtype=np.float32)),
        "onesc": np.ones((128, 128), dtype=np.float32),
        "identf": np.eye(128, dtype=np.float32),
        "ptt2": ptt2.astype(bf),
        "permw": permw_np,
    }
    in_maps = []
    for e in range(E):
        sel = np.zeros((128, E), dtype=np.float32)
        sel[:, e] = 1.0
        m = dict(base)
        m["sel8"] = sel
        m["w1r"] = np.ascontiguousarray(
            np.asarray(W1[e]).reshape(NH, 128, NF, 128)
            .transpose(2, 1, 0, 3)).astype(bf)
        m["w3r"] = np.ascontiguousarray(
            np.asarray(W3[e]).reshape(NH, 128, NF, 128)
            .transpose(2, 1, 0, 3)).astype(bf)
        m["w2r"] = np.asarray(W2[e]).astype(bf)
        in_maps.append(m)
    return in_maps


def _host_counts(xf, Wr):
    """Per-expert routed token counts and the minimum top2/top3 logit gap
    (router replicated on host; used only to pick a safe compiled
    capacity). A gap well above fp32 accumulation noise means the device
    router provably selects the same experts, so no capacity margin is
    needed."""
    logits = xf @ np.asarray(Wr, dtype=np.float32).T
    top2 = np.argsort(-logits, axis=-1, kind="stable")[:, :2]
    srt = np.sort(logits, axis=-1)
    gap = float((srt[:, -2] - srt[:, -3]).min())
    return np.bincount(top2.ravel(), minlength=E), gap


def kernel(x, Wr, W1, W2, W3, trace=False):
    from concourse.bass_utils import run_bass_kernel_spmd

    xf = np.asarray(x, dtype=np.float32).reshape(T, H)
    counts, gap = _host_counts(xf, np.asarray(Wr))
    capf, capg = CAPF, CAPG
    mx = int(counts.max())
    need = mx if gap > 1e-4 else mx + 8
    if need > capf:
        capf = -(-(need + 36) // 64) * 64
        capg = max(capg, -(-capf // 128) * 128)
    nc = _get_nc((T, H, FF, E, capg, capf))
    in_maps = make_in_maps(np.asarray(x), np.asarray(Wr), np.asarray(W1),
                           np.asarray(W2), np.asarray(W3), CAPG=capg)
    res = run_bass_kernel_spmd(nc, in_maps, core_ids=list(range(E)),
                               trace=trace)
    out = np.zeros((T, H), dtype=np.float32)
    for r in res.results:
        yT = np.asarray(r["yT"], dtype=np.float32)      # [H, capf]
        w = np.asarray(r["w16o"], dtype=np.float32).T.ravel()   # [T]
        s = np.asarray(r["s16o"], dtype=np.float32).T.ravel()   # [T]
        sel = s < capf
        si = s[sel].astype(np.int64)
        out[sel] += w[sel, None] * yT[:, si].T
    kernel.last_result = res
    return out.reshape(np.asarray(x).shape)


# revision 79
# speedup vs baseline: 1.0020x; 1.0020x over previous
"""MoE layer (8 experts, top-2, SwiGLU FFN) on 8 Trainium2 NeuronCores.

Strategy: expert parallelism. Each core owns one expert's weights (bf16)
and redundantly computes the fp32 router (cheap). Token dispatch is done
with the gpsimd dma_gather instruction (indirect DMA straight into the
transposed [h, slot] activation layout) instead of a one-hot matmul, so
the tensor engine only runs the FFN itself. The device returns the
per-slot expert outputs yT[H, CAP] plus the routing info (slot + weight
per token); the host applies the routing weights and scatter-adds the 8
cores' slots back to token order (the expert "combine").
"""

import numpy as np
import ml_dtypes

import concourse.mybir as mybir
import concourse.tile as tile
from concourse import bacc
from concourse import library_config

F32 = mybir.dt.float32
BF16 = mybir.dt.bfloat16
I16 = mybir.dt.int16
F16 = mybir.dt.float16
AT = mybir.ActivationFunctionType
OP = mybir.AluOpType

# Problem sizes (fixed by the reference model)
B, S, H, FF, E = 2, 1024, 1024, 4096, 8
T = B * S                       # 2048 tokens
CAPG = 640                      # slot space (multiple of 128; max count 540)
CAPF = 540                      # FFN capacity actually computed (2x270 chunks)
BIG = 65536.0                   # "no slot" marker; exact fp32 round-trip
WHOLD = 0.026                   # ms: hold FFN weight stream off the DMA
                                # queues until the router's xT load is done


def _chunks(total, step):
    out, o = [], 0
    while o < total:
        out.append((o, min(step, total - o)))
        o += step
    return out


def build_nc(T=T, H=H, FF=FF, E=E, CAPG=CAPG, CAPF=CAPF):
    NT, NH, NF = T // 128, H // 128, FF // 128
    NCG = CAPG // 128           # slot tiles (128-wide) in slot space
    # equal-split FFN capacity chunks <=512 keep matmuls compute-bound
    ncch = (CAPF + 511) // 512
    CCH = _chunks(CAPF, -(-CAPF // ncch))

    nc = bacc.Bacc("TRN2", target_bir_lowering=False, debug=False)

    xT = nc.dram_tensor("xT", [H, T], F32, kind="ExternalInput")
    xtok = nc.dram_tensor("xtok", [T, H], BF16, kind="ExternalInput")
    wrT = nc.dram_tensor("wrT", [128, H // 128, E], F32, kind="ExternalInput")
    sel8 = nc.dram_tensor("sel8", [128, E], F32, kind="ExternalInput")
    w1r = nc.dram_tensor("w1r", [NF, 128, NH, 128], BF16, kind="ExternalInput")
    w3r = nc.dram_tensor("w3r", [NF, 128, NH, 128], BF16, kind="ExternalInput")
    w2r = nc.dram_tensor("w2r", [FF, H], BF16, kind="ExternalInput")
    iotaC = nc.dram_tensor("iotaC", [128, CAPG], F16, kind="ExternalInput")
    uincl = nc.dram_tensor("uincl", [128, 128], F32, kind="ExternalInput")
    onesc = nc.dram_tensor("onesc", [128, 128], F32, kind="ExternalInput")
    identf = nc.dram_tensor("identf", [128, 128], F32, kind="ExternalInput")
    # lhsT columns for the slot->token matmul: col 2*tt = partition index p,
    # col 2*tt+1 = tt*128 (both exact in bf16)
    ptt2 = nc.dram_tensor("ptt2", [128, 2 * NT], BF16, kind="ExternalInput")
    # partition-permutation matrices for the idx 16-wrap:
    # permw[c, k, p] = 1 iff c == k*16 + p%16
    permw = nc.dram_tensor("permw", [128, 8, 128], F32, kind="ExternalInput")

    yTo = nc.dram_tensor("yT", [H, CAPF], BF16, kind="ExternalOutput")
    w16o = nc.dram_tensor("w16o", [128, NT], F32, kind="ExternalOutput")
    s16o = nc.dram_tensor("s16o", [128, NT], F32, kind="ExternalOutput")

    with tile.TileContext(nc) as tc:
        with (
            tc.tile_pool(name="const", bufs=1) as constp,
            tc.tile_pool(name="pers", bufs=1) as pers,
            tc.tile_pool(name="stream", bufs=2) as streamp,
            tc.tile_pool(name="wstream", bufs=4) as wstream,
            tc.tile_pool(name="outp", bufs=4) as outp,
        ):
            nc.gpsimd.load_library(library_config.mlp)

            # ---- constants ----
            # only the router-critical wrT goes first; the rest are issued
            # after the router's xT DMAs so they don't delay the front
            wrT_sb = constp.tile([128, NH, E], F32)
            nc.sync.dma_start(wrT_sb, wrT[:])
            sel_sb = constp.tile([128, E], F32)
            nc.sync.dma_start(sel_sb, sel8[:])
            # dummy op: pull the scalar engine's Exp table load (~1.3us)
            # into the idle xT-load window instead of paying it on the
            # top-2 critical path at the first real Exp
            dume = streamp.tile([1, 2], F32, tag="dume")
            nc.scalar.activation(dume, sel_sb[0:1, 0:2], AT.Exp)
            iota_sb = constp.tile([128, CAPG], F16)
            u_sb = constp.tile([128, 128], F32)
            ones_sb = constp.tile([128, 128], F32)
            idf_sb = constp.tile([128, 128], F32)
            ptt2_sb = constp.tile([128, 2 * NT], BF16)
            permw_sb = constp.tile([128, 8, 128], F32)

            le16 = pers.tile([128, NT], F32)     # own-expert logit
            max8_sb = pers.tile([128, NT, 8], F32)
            m16 = pers.tile([128, NT], F32)
            w16 = pers.tile([128, NT], F32)
            s16 = pers.tile([128, NT], F32)
            xgT = pers.tile([128, NH, CAPG], BF16)
            hmid = pers.tile([128, NF, CAPF], BF16)
            idxs_sb = pers.tile([128, CAPG // 16], I16)

            with tc.tile_pool(name="ps_small", bufs=6,
                              space="PSUM") as ps_small:
                # ---- router (fp32): logitsT[E, T], WrT stationary ----
                # full xT rows per DMA (8KB/partition) for DMA efficiency;
                # token chunks become interleaved psum groups
                lgT_sb = pers.tile([E, T], F32)
                TCH = _chunks(T, 512)
                ps_lrs = [ps_small.tile([128, 512], F32, tag="small",
                                        name=f"pslr{i}")
                          for i in range(len(TCH))]
                # token-chunk-major load order: all 8 h-rows of the first
                # token half arrive at ~50% of the 8.4MB load, so the
                # per-tile logit post-processing (transpose/max8/mask)
                # overlaps the DMA tail instead of serializing after it.
                # 1024-column pieces: narrower splits shrink the
                # per-partition descriptor below 4KB (DMA efficiency);
                # bigger ones pin too much traffic on one engine.
                with tc.tile_pool(name="xtfp", bufs=1) as xtfp:
                    xtfs = [xtfp.tile([128, T], F32, tag=f"x{h}",
                                      name=f"xtf{h}")
                            for h in range(NH)]
                    for pch in range(0, T, 1024):
                        for ht in range(NH):
                            nc.sync.dma_start(
                                xtfs[ht][:, pch:pch + 1024],
                                xT[ht * 128:(ht + 1) * 128, pch:pch + 1024])
                        if pch == 0:
                            # non-critical const loads, after first xT
                            nc.sync.dma_start(iota_sb, iotaC[:])
                            nc.sync.dma_start(u_sb, uincl[:])
                            nc.sync.dma_start(ones_sb, onesc[:])
                            nc.sync.dma_start(idf_sb, identf[:])
                            nc.sync.dma_start(ptt2_sb, ptt2[:])
                            nc.sync.dma_start(permw_sb, permw[:])
                        for i in (pch // 512, pch // 512 + 1):
                            to, ts_ = TCH[i]
                            for ht in range(NH):
                                nc.tensor.matmul(ps_lrs[i][:E, :ts_],
                                                 lhsT=wrT_sb[:, ht, :],
                                                 rhs=xtfs[ht][:, to:to + ts_],
                                                 start=(ht == 0),
                                                 stop=(ht == NH - 1))
                for i, (to, ts_) in enumerate(TCH):
                    nc.scalar.copy(lgT_sb[:, to:to + ts_],
                                   ps_lrs[i][:E, :ts_])
                # prefetch the first FFN1 weight tiles ahead of the
                # dispatch phase
                pre_w = []
                for ft in range(2):
                    w1t = wstream.tile([128, NH, 128], BF16, tag="w1t")
                    nc.sync.dma_start(w1t, w1r[ft])
                    w3t = wstream.tile([128, NH, 128], BF16, tag="w3t")
                    nc.sync.dma_start(w3t, w3r[ft])
                    pre_w.append((w1t, w3t))
                # transpose logitsT back to [token_p, E] per tile
                for tt in range(NT):
                    ps_lt = ps_small.tile([128, 128], F32, tag="small")
                    nc.tensor.transpose(
                        ps_lt[:, :E],
                        lgT_sb[:, tt * 128:(tt + 1) * 128],
                        idf_sb[:E, :E])
                    lg = streamp.tile([128, E], F32, tag="lg")
                    nc.scalar.copy(lg, ps_lt[:, :E])
                    nc.vector.max(max8_sb[:, tt, :], lg)
                    tmp8 = streamp.tile([128, E], F32, tag="tmp8")
                    nc.vector.tensor_mul(tmp8, lg, sel_sb)
                    nc.vector.tensor_reduce(
                        le16[:, tt:tt + 1], tmp8, mybir.AxisListType.X,
                        OP.add)
                    # m16 computed per tile inside the DMA-paced router
                    # loop: the cumsum below then starts immediately after
                    # the last tile, with no batched chain in front of it
                    nc.vector.tensor_tensor(
                        m16[:, tt:tt + 1], le16[:, tt:tt + 1],
                        max8_sb[:, tt, 1:2], OP.is_ge)

                # ---- slot assignment: cumsum of mask over tokens ----
                ps_cs = ps_small.tile([128, 128], F32, tag="small")
                nc.tensor.matmul(ps_cs[:, :NT], lhsT=u_sb, rhs=m16,
                                 start=True, stop=True)
                ps_tot = ps_small.tile([128, 128], F32, tag="small")
                nc.tensor.matmul(ps_tot[:, :NT], lhsT=ones_sb, rhs=m16,
                                 start=True, stop=True)
                tot_sb = pers.tile([128, NT], F32)
                nc.scalar.copy(tot_sb, ps_tot[:, :NT])
                isc1 = pers.tile([128, NT], F32)
                nc.vector.tensor_tensor_scan(
                    out=isc1, data0=tot_sb, data1=ones_sb[:, :NT],
                    initial=-1.0, op0=OP.add, op1=OP.mult)
                carrym1 = pers.tile([128, NT], F32)
                nc.vector.tensor_sub(carrym1, isc1, tot_sb)
                s_a = pers.tile([128, NT], F32)
                nc.vector.tensor_tensor(s_a, ps_cs[:, :NT], carrym1, OP.add)
                # s16 = m16 ? s_a : BIG  ==  (s_a - BIG)*m16 + BIG
                # (exact fp32 arithmetic, fused into two DVE ops)
                nc.vector.scalar_tensor_tensor(s_a, s_a, BIG, m16,
                                               OP.subtract, OP.mult)
                nc.vector.tensor_scalar(s16, s_a, BIG, None, OP.add)
                nc.sync.dma_start(s16o[:], s16)

                # ---- slot -> token index vector (exact int arithmetic) ----
                # one-hot dispatch matrix St[tok_p, tile, slot], then
                # tok(c) = sum_t p*St + sum_t (tt*128)*St via 2-col matmuls
                with tc.tile_pool(name="stp", bufs=1) as stp:
                    St = stp.tile([128, NT, CAPG], BF16)
                    # slots beyond CAPF never occur (max count <= CAPF by
                    # the host capacity check), so the one-hot / slot->token
                    # work only covers CAPF of the CAPG slot space
                    for tt in range(NT):
                        # fp16 iota halves the dominant DVE read
                        # traffic on this serial chain (ids exact in fp16)
                        nc.vector.tensor_scalar(
                            St[:, tt, :CAPF], iota_sb[:, :CAPF],
                            s16[:, tt:tt + 1], None, OP.is_equal)
                    TKCH = _chunks(CAPF, 512)
                    ps_toks = [ps_small.tile([2, 512], F32, tag="small",
                                             name=f"pstok{i}")
                               for i in range(len(TKCH))]
                    for tt in range(NT):
                        for i, (co, cs) in enumerate(TKCH):
                            nc.tensor.matmul(
                                ps_toks[i][:, :cs],
                                lhsT=ptt2_sb[:, 2 * tt:2 * tt + 2],
                                rhs=St[:, tt, co:co + cs],
                                start=(tt == 0), stop=(tt == NT - 1))
                    tok2 = pers.tile([2, CAPG], F32)
                    # the whole dead tail must be zeroed: NaN garbage in
                    # any tokc row poisons the permutation matmuls
                    # (0*NaN=NaN spreads across the wrap column). Split at
                    # the 16-boundary into two known-legal memset widths.
                    pad16 = -(-CAPF // 16) * 16
                    if pad16 > CAPF:
                        nc.vector.memset(tok2[:, CAPF:pad16], 0.0)
                    if CAPG > pad16:
                        nc.vector.memset(tok2[:, pad16:], 0.0)
                    for i, (co, cs) in enumerate(TKCH):
                        nc.scalar.copy(tok2[:, co:co + cs],
                                       ps_toks[i][:, :cs])
                # transpose [2, CAPG] -> [128, NCG, 2]; add the two columns.
                # All transposes land in one PSUM tile so the copy and the
                # strided add are single instructions (this chain is
                # instruction-latency-bound, not throughput-bound)
                tokc = pers.tile([128, NCG], F32)
                ps_tt = ps_small.tile([128, NCG, 2], F32, tag="small")
                for ct in range(NCG):
                    nc.tensor.transpose(
                        ps_tt[:, ct, :], tok2[:, ct * 128:(ct + 1) * 128],
                        idf_sb[:2, :2])
                tk2 = streamp.tile([128, NCG, 2], F32, tag="tk2")
                nc.scalar.copy(tk2, ps_tt)
                nc.vector.tensor_tensor(
                    tokc, tk2[:, :, 0], tk2[:, :, 1], OP.add)
                # wrap to the gpsimd idx layout ([16, CAPG/16] wrapped,
                # replicated on all 128 partitions) ON-CHIP via 8 constant
                # partition-permutation matmuls (exact fp32 0/1). A DRAM
                # round-trip here would ride the DMA engines, which are
                # saturated by the weight stream at this point (~30us stall)
                for k in range(8):
                    ps_pk = ps_small.tile([128, NCG], F32, tag="small")
                    nc.tensor.matmul(ps_pk, lhsT=permw_sb[:, k, :],
                                     rhs=tokc, start=True, stop=True)
                    # cast straight from PSUM into the strided int16
                    # slots, on the otherwise-idle scalar engine
                    nc.scalar.copy(idxs_sb[:, k::8], ps_pk)
                # slots >= CAPF get idx -1: the gather stops at the last
                # non-negative index, skipping ~15% of descriptor
                # generation and transfer for the dead slot-space tail
                nwrap = -(-CAPF // 16)
                nc.vector.memset(idxs_sb[:, nwrap:], -1.0)

                # ---- top-2 softmax weights (off the critical path: they
                # only feed the w16o output, so they run while the gather
                # descriptor generation proceeds on gpsimd) ----
                l1 = max8_sb[:, :, 0]
                l2 = max8_sb[:, :, 1]
                d_e = pers.tile([128, NT], F32)
                nc.vector.tensor_sub(d_e, le16, l1)
                e_e = pers.tile([128, NT], F32)
                nc.scalar.activation(e_e, d_e, AT.Exp)
                d_2 = pers.tile([128, NT], F32)
                nc.vector.tensor_sub(d_2, l2, l1)
                e_2 = pers.tile([128, NT], F32)
                # (sigmoid would be one op, but switching the scalar
                # engine's activation table Exp->Sigmoid costs ~1.3us)
                nc.scalar.activation(e_2, d_2, AT.Exp)
                nc.vector.tensor_scalar_add(e_2, e_2, 1.0)
                rden = pers.tile([128, NT], F32)
                nc.vector.reciprocal(rden, e_2)
                nc.vector.tensor_mul(w16, e_e, rden)
                nc.vector.tensor_mul(w16, w16, m16)
                nc.sync.dma_start(w16o[:], w16)
                # dummy op: pull the scalar engine's Sigmoid table load
                # (~1.3us) into the idle dispatch window instead of paying
                # it at FFN1's first real sigmoid
                dumw = streamp.tile([1, 2], F32, tag="dumw")
                nc.scalar.activation(dumw, w16[0:1, 0:2], AT.Sigmoid)

            # ---- token gather: xgT[h, c] = x[tok(c), h] via indirect DMA --
            nc.gpsimd.dma_gather(xgT[:], xtok[:, :], idxs_sb[:],
                                 CAPG, nwrap * 16, H, transpose=True)

            # ---- FFN part 1 + W2 residency prefetch ----
            with tc.tile_pool(name="w2pool", bufs=1) as w2pool:
                w2res = w2pool.tile([128, NF, H], BF16)
                w2rr = w2r.rearrange("(n p) h -> p n h", p=128)
                with (
                    tc.tile_pool(name="ps_gate", bufs=2,
                                 space="PSUM") as ps_gate,
                    tc.tile_pool(name="ps_up", bufs=2, space="PSUM") as ps_up,
                    # ps_y opened alongside (7 of 8 banks total): a
                    # close/reopen barrier between FFN1 and FFN2 costs ~1.2us
                    tc.tile_pool(name="ps_y", bufs=3, space="PSUM") as ps_y,
                ):
                    for ft in range(NF):
                        if ft < len(pre_w):
                            w1t, w3t = pre_w[ft]
                        else:
                            w1t = wstream.tile([128, NH, 128], BF16,
                                               tag="w1t")
                            w3t = wstream.tile([128, NH, 128], BF16,
                                               tag="w3t")
                            # Gate the stream behind the token gather: a
                            # dummy gpsimd write into the target tile reads
                            # xgT (ready only once the gather DMA landed)
                            # and the DMA's WAW dep on it holds the weight
                            # traffic off the DMA engines until then.
                            # Ungated, weights steal ~half the bandwidth
                            # from the router's xT load and the gather,
                            # pushing FFN1's start out ~28us. The stream
                            # still finishes well before FFN2 needs W2.
                            if ft < 6:
                                nc.gpsimd.tensor_copy(w1t[0:E, 0, 0:1],
                                                      xgT[0:E, 0, 0:1])
                                nc.gpsimd.tensor_copy(w3t[0:E, 0, 0:1],
                                                      xgT[0:E, 0, 0:1])
                            nc.sync.dma_start(w1t, w1r[ft])
                            nc.sync.dma_start(w3t, w3r[ft])
                        nc.gpsimd.tensor_copy(w2res[0:E, ft, 0:1],
                                              xgT[0:E, 0, 0:1])
                        # interleave the W2 residency load with the stream
                        nc.sync.dma_start(w2res[:, ft, :],
                                          w2rr[:, ft, :])
                        for (co, cs) in CCH:
                            psg = ps_gate.tile([128, 512], F32, tag="gate")
                            psu = ps_up.tile([128, 512], F32, tag="up")
                            for ht in range(NH):
                                nc.tensor.matmul(
                                    psg[:, :cs], lhsT=w1t[:, ht, :],
                                    rhs=xgT[:, ht, co:co + cs],
                                    start=(ht == 0), stop=(ht == NH - 1))
                            for ht in range(NH):
                                nc.tensor.matmul(
                                    psu[:, :cs], lhsT=w3t[:, ht, :],
                                    rhs=xgT[:, ht, co:co + cs],
                                    start=(ht == 0), stop=(ht == NH - 1))
                            sil = streamp.tile([128, 512], F32, tag="sil")
                            nc.scalar.activation(sil[:, :cs], psg[:, :cs],
                                                 AT.Sigmoid)
                            tmp = streamp.tile([128, 512], F32, tag="ftmp")
                            nc.vector.tensor_mul(tmp[:, :cs], sil[:, :cs],
                                                 psu[:, :cs])
                            nc.vector.tensor_mul(hmid[:, ft, co:co + cs],
                                                 tmp[:, :cs], psg[:, :cs])

                    # ---- FFN part 2: yT[h, c] = sum_f W2[f, h] hmid[f, c] --
                    yTr = yTo.rearrange("(n p) c -> p n c", p=128)
                    ngrp = len(CCH) * NH
                    for gi, ((co, cs), ht) in enumerate(
                            (c, h) for c in CCH for h in range(NH)):
                        psy = ps_y.tile([128, 512], F32, tag="y")
                        for ft in range(NF):
                            nc.tensor.matmul(
                                psy[:, :cs],
                                lhsT=w2res[:, ft,
                                           ht * 128:(ht + 1) * 128],
                                rhs=hmid[:, ft, co:co + cs],
                                start=(ft == 0), stop=(ft == NF - 1))
                        ysb = outp.tile([128, 512], BF16, tag="ysb")
                        nc.scalar.copy(ysb[:, :cs], psy[:, :cs])
                        nc.sync.dma_start(yTr[:, ht, co:co + cs],
                                          ysb[:, :cs])

    nc.compile()
    return nc


_NC_CACHE = {}


def _get_nc(key=(T, H, FF, E, CAPG, CAPF)):
    if key not in _NC_CACHE:
        _NC_CACHE[key] = build_nc(*key)
    return _NC_CACHE[key]


def make_in_maps(x, Wr, W1, W2, W3, T=T, H=H, FF=FF, E=E, CAPG=CAPG):
    NT, NH, NF = T // 128, H // 128, FF // 128
    bf = ml_dtypes.bfloat16
    xf = np.ascontiguousarray(x.reshape(T, H)).astype(np.float32)
    ptt2 = np.zeros((128, 2 * NT), dtype=np.float32)
    ptt2[:, 0::2] = np.arange(128, dtype=np.float32)[:, None]
    ptt2[:, 1::2] = 128.0 * np.arange(NT, dtype=np.float32)[None, :]
    permw_np = np.zeros((128, 8, 128), dtype=np.float32)
    for k in range(8):
        for p in range(128):
            permw_np[k * 16 + p % 16, k, p] = 1.0
    base = {
        "xT": np.ascontiguousarray(xf.T),
        "xtok": xf.astype(bf),
        "wrT": np.ascontiguousarray(
            np.asarray(Wr, dtype=np.float32).T.reshape(H // 128, 128, -1)
            .transpose(1, 0, 2)),
        "iotaC": np.ascontiguousarray(
            np.tile(np.arange(CAPG, dtype=np.float16), (128, 1))),
        "uincl": np.triu(np.ones((128, 128), dtype=np.float32)),
        "onesc": np.ones((128, 128), dtype=np.float32),
        "identf": np.eye(128, dtype=np.float32),
        "ptt2": ptt2.astype(bf),
        "permw": permw_np,
    }
    in_maps = []
    for e in range(E):
        sel = np.zeros((128, E), dtype=np.float32)
        sel[:, e] = 1.0
        m = dict(base)
        m["sel8"] = sel
        m["w1r"] = np.ascontiguousarray(
            np.asarray(W1[e]).reshape(NH, 128, NF, 128)
            .transpose(2, 1, 0, 3)).astype(bf)
        m["w3r"] = np.ascontiguousarray(
            np.asarray(W3[e]).reshape(NH, 128, NF, 128)
            .transpose(2, 1, 0, 3)).astype(bf)
        m["w2r"] = np.asarray(W2[e]).astype(bf)
        in_maps.append(m)
    return in_maps


def _host_counts(xf, Wr):
    """Per-expert routed token counts and the minimum top2/top3 logit gap
    (router replicated on host; used only to pick a safe compiled
    capacity). A gap well above fp32 accumulation noise means the device
    router provably selects the same experts, so no capacity margin is
    needed."""
    logits = xf @ np.asarray(Wr, dtype=np.float32).T
    top2 = np.argsort(-logits, axis=-1, kind="stable")[:, :2]
    srt = np.sort(logits, axis=-1)
    gap = float((srt[:, -2] - srt[:, -3]).min())
    return np.bincount(top2.ravel(), minlength=E), gap


def kernel(x, Wr, W1, W2, W3, trace=False):
    from concourse.bass_utils import run_bass_kernel_spmd

    xf = np.asarray(x, dtype=np.float32).reshape(T, H)
    counts, gap = _host_counts(xf, np.asarray(Wr))
    capf, capg = CAPF, CAPG
    mx = int(counts.max())
    need = mx if gap > 1e-4 else mx + 8
    if need > capf:
        capf = -(-(need + 36) // 64) * 64
        capg = max(capg, -(-capf // 128) * 128)
    nc = _get_nc((T, H, FF, E, capg, capf))
    in_maps = make_in_maps(np.asarray(x), np.asarray(Wr), np.asarray(W1),
                           np.asarray(W2), np.asarray(W3), CAPG=capg)
    res = run_bass_kernel_spmd(nc, in_maps, core_ids=list(range(E)),
                               trace=trace)
    out = np.zeros((T, H), dtype=np.float32)
    for r in res.results:
        yT = np.asarray(r["yT"], dtype=np.float32)      # [H, capf]
        w = np.asarray(r["w16o"], dtype=np.float32).T.ravel()   # [T]
        s = np.asarray(r["s16o"], dtype=np.float32).T.ravel()   # [T]
        sel = s < capf
        si = s[sel].astype(np.int64)
        out[sel] += w[sel, None] * yT[:, si].T
    kernel.last_result = res
    return out.reshape(np.asarray(x).shape)


# revision 80
# speedup vs baseline: 1.0058x; 1.0038x over previous
"""MoE layer (8 experts, top-2, SwiGLU FFN) on 8 Trainium2 NeuronCores.

Strategy: expert parallelism. Each core owns one expert's weights (bf16)
and redundantly computes the fp32 router (cheap). Token dispatch is done
with the gpsimd dma_gather instruction (indirect DMA straight into the
transposed [h, slot] activation layout) instead of a one-hot matmul, so
the tensor engine only runs the FFN itself. The device returns the
per-slot expert outputs yT[H, CAP] plus the routing info (slot + weight
per token); the host applies the routing weights and scatter-adds the 8
cores' slots back to token order (the expert "combine").
"""

import numpy as np
import ml_dtypes

import concourse.mybir as mybir
import concourse.tile as tile
from concourse import bacc
from concourse import library_config

F32 = mybir.dt.float32
BF16 = mybir.dt.bfloat16
I16 = mybir.dt.int16
F16 = mybir.dt.float16
AT = mybir.ActivationFunctionType
OP = mybir.AluOpType

# Problem sizes (fixed by the reference model)
B, S, H, FF, E = 2, 1024, 1024, 4096, 8
T = B * S                       # 2048 tokens
CAPG = 640                      # slot space (multiple of 128; max count 540)
CAPF = 540                      # FFN capacity actually computed (2x270 chunks)
BIG = 65536.0                   # "no slot" marker; exact fp32 round-trip
WHOLD = 0.026                   # ms: hold FFN weight stream off the DMA
                                # queues until the router's xT load is done


def _chunks(total, step):
    out, o = [], 0
    while o < total:
        out.append((o, min(step, total - o)))
        o += step
    return out


def build_nc(T=T, H=H, FF=FF, E=E, CAPG=CAPG, CAPF=CAPF):
    NT, NH, NF = T // 128, H // 128, FF // 128
    NCG = CAPG // 128           # slot tiles (128-wide) in slot space
    # equal-split FFN capacity chunks <=512 keep matmuls compute-bound
    ncch = (CAPF + 511) // 512
    CCH = _chunks(CAPF, -(-CAPF // ncch))

    nc = bacc.Bacc("TRN2", target_bir_lowering=False, debug=False)

    xT = nc.dram_tensor("xT", [H, T], F32, kind="ExternalInput")
    xtok = nc.dram_tensor("xtok", [T, H], BF16, kind="ExternalInput")
    wrT = nc.dram_tensor("wrT", [128, H // 128, E], F32, kind="ExternalInput")
    sel8 = nc.dram_tensor("sel8", [128, E], F32, kind="ExternalInput")
    w1r = nc.dram_tensor("w1r", [NF, 128, NH, 128], BF16, kind="ExternalInput")
    w3r = nc.dram_tensor("w3r", [NF, 128, NH, 128], BF16, kind="ExternalInput")
    w2r = nc.dram_tensor("w2r", [FF, H], BF16, kind="ExternalInput")
    iotaC = nc.dram_tensor("iotaC", [128, CAPG], F16, kind="ExternalInput")
    uincl = nc.dram_tensor("uincl", [128, 128], F32, kind="ExternalInput")
    onesc = nc.dram_tensor("onesc", [128, 128], F32, kind="ExternalInput")
    identf = nc.dram_tensor("identf", [128, 128], F32, kind="ExternalInput")
    # lhsT columns for the slot->token matmul: col 2*tt = partition index p,
    # col 2*tt+1 = tt*128 (both exact in bf16)
    ptt2 = nc.dram_tensor("ptt2", [128, 2 * NT], BF16, kind="ExternalInput")
    # partition-permutation matrices for the idx 16-wrap:
    # permw[c, k, p] = 1 iff c == k*16 + p%16
    permw = nc.dram_tensor("permw", [128, 8, 128], F32, kind="ExternalInput")

    yTo = nc.dram_tensor("yT", [H, CAPF], BF16, kind="ExternalOutput")
    w16o = nc.dram_tensor("w16o", [128, NT], F32, kind="ExternalOutput")
    s16o = nc.dram_tensor("s16o", [128, NT], F32, kind="ExternalOutput")

    with tile.TileContext(nc) as tc:
        with (
            tc.tile_pool(name="const", bufs=1) as constp,
            tc.tile_pool(name="pers", bufs=1) as pers,
            tc.tile_pool(name="stream", bufs=2) as streamp,
            tc.tile_pool(name="wstream", bufs=4) as wstream,
            tc.tile_pool(name="outp", bufs=4) as outp,
        ):
            nc.gpsimd.load_library(library_config.mlp)

            # ---- constants ----
            # only the router-critical wrT goes first; the rest are issued
            # after the router's xT DMAs so they don't delay the front
            wrT_sb = constp.tile([128, NH, E], F32)
            nc.sync.dma_start(wrT_sb, wrT[:])
            sel_sb = constp.tile([128, E], F32)
            nc.sync.dma_start(sel_sb, sel8[:])
            # dummy op: pull the scalar engine's Exp table load (~1.3us)
            # into the idle xT-load window instead of paying it on the
            # top-2 critical path at the first real Exp
            dume = streamp.tile([1, 2], F32, tag="dume")
            nc.scalar.activation(dume, sel_sb[0:1, 0:2], AT.Exp)
            iota_sb = constp.tile([128, CAPG], F16)
            u_sb = constp.tile([128, 128], F32)
            ones_sb = constp.tile([128, 128], F32)
            idf_sb = constp.tile([128, 128], F32)
            ptt2_sb = constp.tile([128, 2 * NT], BF16)
            permw_sb = constp.tile([128, 8, 128], F32)

            le16 = pers.tile([128, NT], F32)     # own-expert logit
            max8_sb = pers.tile([128, NT, 8], F32)
            m16 = pers.tile([128, NT], F32)
            w16 = pers.tile([128, NT], F32)
            s16 = pers.tile([128, NT], F32)
            xgT = pers.tile([128, NH, CAPG], BF16)
            hmid = pers.tile([128, NF, CAPF], BF16)
            idxs_sb = pers.tile([128, CAPG // 16], I16)

            with tc.tile_pool(name="ps_small", bufs=8,
                              space="PSUM") as ps_small:
                # ---- router (fp32): logitsT[E, T], WrT stationary ----
                # full xT rows per DMA (8KB/partition) for DMA efficiency;
                # token chunks become interleaved psum groups
                lgT_sb = pers.tile([E, T], F32)
                TCH = _chunks(T, 512)
                ps_lrs = [ps_small.tile([128, 512], F32, tag="small",
                                        name=f"pslr{i}")
                          for i in range(len(TCH))]
                # token-chunk-major load order: all 8 h-rows of the first
                # token half arrive at ~50% of the 8.4MB load, so the
                # per-tile logit post-processing (transpose/max8/mask)
                # overlaps the DMA tail instead of serializing after it.
                # 1024-column pieces: narrower splits shrink the
                # per-partition descriptor below 4KB (DMA efficiency);
                # bigger ones pin too much traffic on one engine.
                with tc.tile_pool(name="xtfp", bufs=1) as xtfp:
                    xtfs = [xtfp.tile([128, T], F32, tag=f"x{h}",
                                      name=f"xtf{h}")
                            for h in range(NH)]
                    for pch in range(0, T, 1024):
                        for ht in range(NH):
                            nc.sync.dma_start(
                                xtfs[ht][:, pch:pch + 1024],
                                xT[ht * 128:(ht + 1) * 128, pch:pch + 1024])
                        if pch == 0:
                            # non-critical const loads, after first xT
                            nc.sync.dma_start(iota_sb, iotaC[:])
                            nc.sync.dma_start(u_sb, uincl[:])
                            nc.sync.dma_start(ones_sb, onesc[:])
                            nc.sync.dma_start(idf_sb, identf[:])
                            nc.sync.dma_start(ptt2_sb, ptt2[:])
                            nc.sync.dma_start(permw_sb, permw[:])
                        for i in (pch // 512, pch // 512 + 1):
                            to, ts_ = TCH[i]
                            for ht in range(NH):
                                nc.tensor.matmul(ps_lrs[i][:E, :ts_],
                                                 lhsT=wrT_sb[:, ht, :],
                                                 rhs=xtfs[ht][:, to:to + ts_],
                                                 start=(ht == 0),
                                                 stop=(ht == NH - 1))
                for i, (to, ts_) in enumerate(TCH):
                    nc.scalar.copy(lgT_sb[:, to:to + ts_],
                                   ps_lrs[i][:E, :ts_])
                # prefetch the first FFN1 weight tiles ahead of the
                # dispatch phase
                pre_w = []
                for ft in range(2):
                    w1t = wstream.tile([128, NH, 128], BF16, tag="w1t")
                    nc.sync.dma_start(w1t, w1r[ft])
                    w3t = wstream.tile([128, NH, 128], BF16, tag="w3t")
                    nc.sync.dma_start(w3t, w3r[ft])
                    pre_w.append((w1t, w3t))
                # transpose logitsT back to [token_p, E] per tile
                for tt in range(NT):
                    ps_lt = ps_small.tile([128, 128], F32, tag="small")
                    nc.tensor.transpose(
                        ps_lt[:, :E],
                        lgT_sb[:, tt * 128:(tt + 1) * 128],
                        idf_sb[:E, :E])
                    lg = streamp.tile([128, E], F32, tag="lg")
                    nc.scalar.copy(lg, ps_lt[:, :E])
                    nc.vector.max(max8_sb[:, tt, :], lg)
                    tmp8 = streamp.tile([128, E], F32, tag="tmp8")
                    nc.vector.tensor_mul(tmp8, lg, sel_sb)
                    nc.vector.tensor_reduce(
                        le16[:, tt:tt + 1], tmp8, mybir.AxisListType.X,
                        OP.add)
                    # m16 computed per tile inside the DMA-paced router
                    # loop: the cumsum below then starts immediately after
                    # the last tile, with no batched chain in front of it
                    nc.vector.tensor_tensor(
                        m16[:, tt:tt + 1], le16[:, tt:tt + 1],
                        max8_sb[:, tt, 1:2], OP.is_ge)

                # ---- slot assignment: cumsum of mask over tokens ----
                ps_cs = ps_small.tile([128, 128], F32, tag="small")
                nc.tensor.matmul(ps_cs[:, :NT], lhsT=u_sb, rhs=m16,
                                 start=True, stop=True)
                ps_tot = ps_small.tile([128, 128], F32, tag="small")
                nc.tensor.matmul(ps_tot[:, :NT], lhsT=ones_sb, rhs=m16,
                                 start=True, stop=True)
                tot_sb = pers.tile([128, NT], F32)
                nc.scalar.copy(tot_sb, ps_tot[:, :NT])
                isc1 = pers.tile([128, NT], F32)
                nc.vector.tensor_tensor_scan(
                    out=isc1, data0=tot_sb, data1=ones_sb[:, :NT],
                    initial=-1.0, op0=OP.add, op1=OP.mult)
                carrym1 = pers.tile([128, NT], F32)
                nc.vector.tensor_sub(carrym1, isc1, tot_sb)
                s_a = pers.tile([128, NT], F32)
                nc.vector.tensor_tensor(s_a, ps_cs[:, :NT], carrym1, OP.add)
                # s16 = m16 ? s_a : BIG  ==  (s_a - BIG)*m16 + BIG
                # (exact fp32 arithmetic, fused into two DVE ops)
                nc.vector.scalar_tensor_tensor(s_a, s_a, BIG, m16,
                                               OP.subtract, OP.mult)
                nc.vector.tensor_scalar(s16, s_a, BIG, None, OP.add)
                nc.sync.dma_start(s16o[:], s16)

                # ---- slot -> token index vector (exact int arithmetic) ----
                # one-hot dispatch matrix St[tok_p, tile, slot], then
                # tok(c) = sum_t p*St + sum_t (tt*128)*St via 2-col matmuls
                with tc.tile_pool(name="stp", bufs=1) as stp:
                    St = stp.tile([128, NT, CAPG], BF16)
                    # slots beyond CAPF never occur (max count <= CAPF by
                    # the host capacity check), so the one-hot / slot->token
                    # work only covers CAPF of the CAPG slot space
                    for tt in range(NT):
                        # fp16 iota halves the dominant DVE read
                        # traffic on this serial chain (ids exact in fp16)
                        nc.vector.tensor_scalar(
                            St[:, tt, :CAPF], iota_sb[:, :CAPF],
                            s16[:, tt:tt + 1], None, OP.is_equal)
                    TKCH = _chunks(CAPF, 512)
                    ps_toks = [ps_small.tile([2, 512], F32, tag="small",
                                             name=f"pstok{i}")
                               for i in range(len(TKCH))]
                    for tt in range(NT):
                        for i, (co, cs) in enumerate(TKCH):
                            nc.tensor.matmul(
                                ps_toks[i][:, :cs],
                                lhsT=ptt2_sb[:, 2 * tt:2 * tt + 2],
                                rhs=St[:, tt, co:co + cs],
                                start=(tt == 0), stop=(tt == NT - 1))
                    tok2 = pers.tile([2, CAPG], F32)
                    # the whole dead tail must be zeroed: NaN garbage in
                    # any tokc row poisons the permutation matmuls
                    # (0*NaN=NaN spreads across the wrap column). Split at
                    # the 16-boundary into two known-legal memset widths.
                    pad16 = -(-CAPF // 16) * 16
                    if pad16 > CAPF:
                        nc.vector.memset(tok2[:, CAPF:pad16], 0.0)
                    if CAPG > pad16:
                        nc.vector.memset(tok2[:, pad16:], 0.0)
                    for i, (co, cs) in enumerate(TKCH):
                        nc.scalar.copy(tok2[:, co:co + cs],
                                       ps_toks[i][:, :cs])
                # transpose [2, CAPG] -> [128, NCG, 2]; add the two columns.
                # All transposes land in one PSUM tile so the copy and the
                # strided add are single instructions (this chain is
                # instruction-latency-bound, not throughput-bound)
                tokc = pers.tile([128, NCG], F32)
                ps_tt = ps_small.tile([128, NCG, 2], F32, tag="small")
                for ct in range(NCG):
                    nc.tensor.transpose(
                        ps_tt[:, ct, :], tok2[:, ct * 128:(ct + 1) * 128],
                        idf_sb[:2, :2])
                tk2 = streamp.tile([128, NCG, 2], F32, tag="tk2")
                nc.scalar.copy(tk2, ps_tt)
                nc.vector.tensor_tensor(
                    tokc, tk2[:, :, 0], tk2[:, :, 1], OP.add)
                # wrap to the gpsimd idx layout ([16, CAPG/16] wrapped,
                # replicated on all 128 partitions) ON-CHIP via 8 constant
                # partition-permutation matmuls (exact fp32 0/1). A DRAM
                # round-trip here would ride the DMA engines, which are
                # saturated by the weight stream at this point (~30us stall)
                for k in range(8):
                    ps_pk = ps_small.tile([128, NCG], F32, tag="small")
                    nc.tensor.matmul(ps_pk, lhsT=permw_sb[:, k, :],
                                     rhs=tokc, start=True, stop=True)
                    # cast straight from PSUM into the strided int16
                    # slots, on the otherwise-idle scalar engine
                    nc.scalar.copy(idxs_sb[:, k::8], ps_pk)
                # slots >= CAPF get idx -1: the gather stops at the last
                # non-negative index, skipping ~15% of descriptor
                # generation and transfer for the dead slot-space tail
                nwrap = -(-CAPF // 16)
                nc.vector.memset(idxs_sb[:, nwrap:], -1.0)

                # ---- top-2 softmax weights (off the critical path: they
                # only feed the w16o output, so they run while the gather
                # descriptor generation proceeds on gpsimd) ----
                l1 = max8_sb[:, :, 0]
                l2 = max8_sb[:, :, 1]
                d_e = pers.tile([128, NT], F32)
                nc.vector.tensor_sub(d_e, le16, l1)
                e_e = pers.tile([128, NT], F32)
                nc.scalar.activation(e_e, d_e, AT.Exp)
                d_2 = pers.tile([128, NT], F32)
                nc.vector.tensor_sub(d_2, l2, l1)
                e_2 = pers.tile([128, NT], F32)
                # (sigmoid would be one op, but switching the scalar
                # engine's activation table Exp->Sigmoid costs ~1.3us)
                nc.scalar.activation(e_2, d_2, AT.Exp)
                nc.vector.tensor_scalar_add(e_2, e_2, 1.0)
                rden = pers.tile([128, NT], F32)
                nc.vector.reciprocal(rden, e_2)
                nc.vector.tensor_mul(w16, e_e, rden)
                nc.vector.tensor_mul(w16, w16, m16)
                nc.sync.dma_start(w16o[:], w16)
                # dummy op: pull the scalar engine's Sigmoid table load
                # (~1.3us) into the idle dispatch window instead of paying
                # it at FFN1's first real sigmoid
                dumw = streamp.tile([1, 2], F32, tag="dumw")
                nc.scalar.activation(dumw, w16[0:1, 0:2], AT.Sigmoid)

            # ---- token gather: xgT[h, c] = x[tok(c), h] via indirect DMA --
            nc.gpsimd.dma_gather(xgT[:], xtok[:, :], idxs_sb[:],
                                 CAPG, nwrap * 16, H, transpose=True)

            # ---- FFN part 1 + W2 residency prefetch ----
            with tc.tile_pool(name="w2pool", bufs=1) as w2pool:
                w2res = w2pool.tile([128, NF, H], BF16)
                w2rr = w2r.rearrange("(n p) h -> p n h", p=128)
                with (
                    tc.tile_pool(name="ps_gate", bufs=2,
                                 space="PSUM") as ps_gate,
                    tc.tile_pool(name="ps_up", bufs=2, space="PSUM") as ps_up,
                    # ps_y opened alongside (7 of 8 banks total): a
                    # close/reopen barrier between FFN1 and FFN2 costs ~1.2us
                    tc.tile_pool(name="ps_y", bufs=3, space="PSUM") as ps_y,
                ):
                    for ft in range(NF):
                        if ft < len(pre_w):
                            w1t, w3t = pre_w[ft]
                        else:
                            w1t = wstream.tile([128, NH, 128], BF16,
                                               tag="w1t")
                            w3t = wstream.tile([128, NH, 128], BF16,
                                               tag="w3t")
                            # Gate the stream behind the token gather: a
                            # dummy gpsimd write into the target tile reads
                            # xgT (ready only once the gather DMA landed)
                            # and the DMA's WAW dep on it holds the weight
                            # traffic off the DMA engines until then.
                            # Ungated, weights steal ~half the bandwidth
                            # from the router's xT load and the gather,
                            # pushing FFN1's start out ~28us. The stream
                            # still finishes well before FFN2 needs W2.
                            if ft < 6:
                                nc.gpsimd.tensor_copy(w1t[0:E, 0, 0:1],
                                                      xgT[0:E, 0, 0:1])
                                nc.gpsimd.tensor_copy(w3t[0:E, 0, 0:1],
                                                      xgT[0:E, 0, 0:1])
                            nc.sync.dma_start(w1t, w1r[ft])
                            nc.sync.dma_start(w3t, w3r[ft])
                        nc.gpsimd.tensor_copy(w2res[0:E, ft, 0:1],
                                              xgT[0:E, 0, 0:1])
                        # interleave the W2 residency load with the stream
                        nc.sync.dma_start(w2res[:, ft, :],
                                          w2rr[:, ft, :])
                        for (co, cs) in CCH:
                            psg = ps_gate.tile([128, 512], F32, tag="gate")
                            psu = ps_up.tile([128, 512], F32, tag="up")
                            for ht in range(NH):
                                nc.tensor.matmul(
                                    psg[:, :cs], lhsT=w1t[:, ht, :],
                                    rhs=xgT[:, ht, co:co + cs],
                                    start=(ht == 0), stop=(ht == NH - 1))
                            for ht in range(NH):
                                nc.tensor.matmul(
                                    psu[:, :cs], lhsT=w3t[:, ht, :],
                                    rhs=xgT[:, ht, co:co + cs],
                                    start=(ht == 0), stop=(ht == NH - 1))
                            sil = streamp.tile([128, 512], F32, tag="sil")
                            nc.scalar.activation(sil[:, :cs], psg[:, :cs],
                                                 AT.Sigmoid)
                            tmp = streamp.tile([128, 512], F32, tag="ftmp")
                            nc.vector.tensor_mul(tmp[:, :cs], sil[:, :cs],
                                                 psu[:, :cs])
                            nc.vector.tensor_mul(hmid[:, ft, co:co + cs],
                                                 tmp[:, :cs], psg[:, :cs])

                    # ---- FFN part 2: yT[h, c] = sum_f W2[f, h] hmid[f, c] --
                    yTr = yTo.rearrange("(n p) c -> p n c", p=128)
                    ngrp = len(CCH) * NH
                    for gi, ((co, cs), ht) in enumerate(
                            (c, h) for c in CCH for h in range(NH)):
                        psy = ps_y.tile([128, 512], F32, tag="y")
                        for ft in range(NF):
                            nc.tensor.matmul(
                                psy[:, :cs],
                                lhsT=w2res[:, ft,
                                           ht * 128:(ht + 1) * 128],
                                rhs=hmid[:, ft, co:co + cs],
                                start=(ft == 0), stop=(ft == NF - 1))
                        ysb = outp.tile([128, 512], BF16, tag="ysb")
                        nc.scalar.copy(ysb[:, :cs], psy[:, :cs])
                        nc.sync.dma_start(yTr[:, ht, co:co + cs],
                                          ysb[:, :cs])

    nc.compile()
    return nc


_NC_CACHE = {}


def _get_nc(key=(T, H, FF, E, CAPG, CAPF)):
    if key not in _NC_CACHE:
        _NC_CACHE[key] = build_nc(*key)
    return _NC_CACHE[key]


def make_in_maps(x, Wr, W1, W2, W3, T=T, H=H, FF=FF, E=E, CAPG=CAPG):
    NT, NH, NF = T // 128, H // 128, FF // 128
    bf = ml_dtypes.bfloat16
    xf = np.ascontiguousarray(x.reshape(T, H)).astype(np.float32)
    ptt2 = np.zeros((128, 2 * NT), dtype=np.float32)
    ptt2[:, 0::2] = np.arange(128, dtype=np.float32)[:, None]
    ptt2[:, 1::2] = 128.0 * np.arange(NT, dtype=np.float32)[None, :]
    permw_np = np.zeros((128, 8, 128), dtype=np.float32)
    for k in range(8):
        for p in range(128):
            permw_np[k * 16 + p % 16, k, p] = 1.0
    base = {
        "xT": np.ascontiguousarray(xf.T),
        "xtok": xf.astype(bf),
        "wrT": np.ascontiguousarray(
            np.asarray(Wr, dtype=np.float32).T.reshape(H // 128, 128, -1)
            .transpose(1, 0, 2)),
        "iotaC": np.ascontiguousarray(
            np.tile(np.arange(CAPG, dtype=np.float16), (128, 1))),
        "uincl": np.triu(np.ones((128, 128), dtype=np.float32)),
        "onesc": np.ones((128, 128), dtype=np.float32),
        "identf": np.eye(128, dtype=np.float32),
        "ptt2": ptt2.astype(bf),
        "permw": permw_np,
    }
    in_maps = []
    for e in range(E):
        sel = np.zeros((128, E), dtype=np.float32)
        sel[:, e] = 1.0
        m = dict(base)
        m["sel8"] = sel
        m["w1r"] = np.ascontiguousarray(
            np.asarray(W1[e]).reshape(NH, 128, NF, 128)
            .transpose(2, 1, 0, 3)).astype(bf)
        m["w3r"] = np.ascontiguousarray(
            np.asarray(W3[e]).reshape(NH, 128, NF, 128)
            .transpose(2, 1, 0, 3)).astype(bf)
        m["w2r"] = np.asarray(W2[e]).astype(bf)
        in_maps.append(m)
    return in_maps


def _host_counts(xf, Wr):
    """Per-expert routed token counts and the minimum top2/top3 logit gap
    (router replicated on host; used only to pick a safe compiled
    capacity). A gap well above fp32 accumulation noise means the device
    router provably selects the same experts, so no capacity margin is
    needed."""
    logits = xf @ np.asarray(Wr, dtype=np.float32).T
    top2 = np.argsort(-logits, axis=-1, kind="stable")[:, :2]
    srt = np.sort(logits, axis=-1)
    gap = float((srt[:, -2] - srt[:, -3]).min())
    return np.bincount(top2.ravel(), minlength=E), gap


def kernel(x, Wr, W1, W2, W3, trace=False):
    from concourse.bass_utils import run_bass_kernel_spmd

    xf = np.asarray(x, dtype=np.float32).reshape(T, H)
    counts, gap = _host_counts(xf, np.asarray(Wr))
    capf, capg = CAPF, CAPG
    mx = int(counts.max())
    need = mx if gap > 1e-4 else mx + 8
    if need > capf:
        capf = -(-(need + 36) // 64) * 64
        capg = max(capg, -(-capf // 128) * 128)
    nc = _get_nc((T, H, FF, E, capg, capf))
    in_maps = make_in_maps(np.asarray(x), np.asarray(Wr), np.asarray(W1),
                           np.asarray(W2), np.asarray(W3), CAPG=capg)
    res = run_bass_kernel_spmd(nc, in_maps, core_ids=list(range(E)),
                               trace=trace)
    out = np.zeros((T, H), dtype=np.float32)
    for r in res.results:
        yT = np.asarray(r["yT"], dtype=np.float32)      # [H, capf]
        w = np.asarray(r["w16o"], dtype=np.float32).T.ravel()   # [T]
        s = np.asarray(r["s16o"], dtype=np.float32).T.ravel()   # [T]
        sel = s < capf
        si = s[sel].astype(np.int64)
        out[sel] += w[sel, None] * yT[:, si].T
    kernel.last_result = res
    return out.reshape(np.asarray(x).shape)
